# revision 19
# baseline (speedup 1.0000x reference)
"""DNGPU cell (gated conv recurrence) for Trainium2, data-parallel over batch on 8 cores.

Problem: B=32, L=128, C=192, K=3; 128 sequential steps of
    reset = sigmoid(conv(mem, w_reset) + b_r)
    gate  = sigmoid(conv(mem, w_gate) + b_g)
    cand  = tanh(conv(reset*mem, w_cand) + b_c)
    mem   = gate*shift_right(mem) + (1-gate)*cand

Per-core layout: state held in SBUF as [C partitions, token cols] where
token col = 4 + l*4 + b (l-major, b-minor, 4 zero-pad cols each side).
Conv taps are column-shifted views (tap k reads cols k*4 .. k*4+512) and
shift_right is the view shifted by -4. C=192 splits into A (0:128) and
B (128:192) channel halves; the B tile keeps a +1l-shifted duplicate in
partitions 64:128 so taps 0,1 pack into one K=128 contraction chunk.

V2 over baseline:
- tap-2 B chunks zero-padded to K=128 (K=64 f32r matmuls stream at half rate)
- rB/gB output halves fused into one M=128 group ([gB|rB] packing so the
  gate half aligns with state partitions 0:64 for GpSimd)
- u = gate*shifted runs on GpSimd (DVE was the second bottleneck)
- no keep-warm dummy matmuls (PE idle tail < HAM window after rescheduling)
"""

import numpy as np
from contextlib import ExitStack

import concourse.bacc as bacc
import concourse.tile as tile
from concourse import mybir
from concourse.bass_utils import run_bass_kernel_spmd

B, L, C = 32, 128, 192
NCORES = 8
BLOC = B // NCORES          # 4 batches per core
TOK = BLOC * L              # 512 tokens per core
WPAD = TOK + 8              # 4 zero cols each side
STEPS = 128

F32 = mybir.dt.float32
F32R = mybir.dt.float32r
AF = mybir.ActivationFunctionType
ALU = mybir.AluOpType

# output groups: name -> (conv, out slice) pieces
#   rA = reset couts 0:128, gA = gate couts 0:128,
#   rgB = [gate 128:192 | reset 128:192]  (gate first: aligns with state B rows)
#   cA = cand couts 0:128, cB = cand couts 128:192


def build(steps=STEPS):
    nc = bacc.Bacc("TRN2", target_bir_lowering=False, debug=False,
                   num_devices=NCORES)
    x_d = nc.dram_tensor("x", [BLOC, L, C], F32, kind="ExternalInput").ap()
    w_d = {}
    b_d = {}
    for cv, wn, bn in (("r", "w_reset", "b_reset"),
                       ("g", "w_gate", "b_gate"),
                       ("n", "w_cand", "b_cand")):
        w_d[cv] = nc.dram_tensor(wn, [3, C, C], F32, kind="ExternalInput").ap()
        b_d[cv] = nc.dram_tensor(bn, [C], F32, kind="ExternalInput").ap()
    id_d = nc.dram_tensor("ident", [128, 128], F32, kind="ExternalInput").ap()
    out_d = nc.dram_tensor("out", [BLOC, L, C], F32, kind="ExternalOutput").ap()

    # group -> list of (conv, cout_lo, cout_hi, dst_lo) for weight packing
    GROUPS = {
        "rA": [("r", 0, 128, 0)],
        "gA": [("g", 0, 128, 0)],
        "rgB": [("g", 128, 192, 0), ("r", 128, 192, 64)],
        "cA": [("n", 0, 128, 0)],
        "cB": [("n", 128, 192, 0)],
    }
    GW = {"rA": 128, "gA": 128, "rgB": 128, "cA": 128, "cB": 64}

    with tile.TileContext(nc) as tc, ExitStack() as ctx:
        const = ctx.enter_context(tc.tile_pool(name="const", bufs=1))
        state = ctx.enter_context(tc.tile_pool(name="state", bufs=1))
        act = ctx.enter_context(tc.tile_pool(name="act", bufs=6))
        tmp = ctx.enter_context(tc.tile_pool(name="tmp", bufs=4))
        psum = ctx.enter_context(tc.tile_pool(name="psum", bufs=1, space="PSUM"))

        zf32 = state.tile([128, WPAD], F32, tag="zf32", name="zf32")
        nc.gpsimd.memset(zf32[:], 0.0)

        # --- weights ----------------------------------------------------
        # wA[G][k]: [128, GW] cinA tap k;  wp[G]: [128, GW] = [cinB tap0; cinB tap1]
        # w2[G]: [128, GW] = [cinB tap2; zeros]
        wA = {}
        wp = {}
        w2 = {}
        for g, pieces in GROUPS.items():
            for k in range(3):
                t = const.tile([128, GW[g]], F32R, tag=f"wA{g}{k}")
                for cv, lo, hi, dst in pieces:
                    nc.gpsimd.dma_start(t[:, dst:dst + hi - lo],
                                        w_d[cv][k, 0:128, lo:hi])
                wA[g, k] = t
            t = const.tile([128, GW[g]], F32R, tag=f"wp{g}", name=f"wp{g}")
            for cv, lo, hi, dst in pieces:
                nc.gpsimd.dma_start(t[0:64, dst:dst + hi - lo],
                                    w_d[cv][0, 128:192, lo:hi])
                nc.gpsimd.dma_start(t[64:128, dst:dst + hi - lo],
                                    w_d[cv][1, 128:192, lo:hi])
            wp[g] = t
            t = const.tile([128, GW[g]], F32R, tag=f"w2{g}", name=f"w2{g}")
            nc.vector.tensor_copy(t[64:128, :], zf32[64:128, 0:GW[g]])
            for cv, lo, hi, dst in pieces:
                nc.gpsimd.dma_start(t[0:64, dst:dst + hi - lo],
                                    w_d[cv][2, 128:192, lo:hi])
            w2[g] = t

        bias = {}
        for g, pieces in GROUPS.items():
            if g.startswith("c"):
                continue
            t = const.tile([GW[g], 1], F32, tag=f"b{g}")
            for cv, lo, hi, dst in pieces:
                nc.sync.dma_start(t[dst:dst + hi - lo, 0], b_d[cv][lo:hi])
            bias[g] = t
        bias_cA = const.tile([128, 1], F32, tag="bcA")
        nc.sync.dma_start(bias_cA[:, 0], b_d["n"][0:128])
        bias_cB = const.tile([64, 1], F32, tag="bcB")
        nc.sync.dma_start(bias_cB[:, 0], b_d["n"][128:192])

        ident = const.tile([128, 128], F32, tag="ident")
        nc.sync.dma_start(ident[:], id_d)
        identr = const.tile([128, 128], F32R, tag="identr")
        nc.gpsimd.dma_start(identr[:], id_d)

        # --- state tiles -------------------------------------------------
        mem = {}
        for i in range(2):
            mem[i, 0] = state.tile([128, WPAD], F32R, tag=f"memA{i}", name=f"memA{i}")
            mem[i, 1] = state.tile([128, WPAD], F32R, tag=f"memB{i}", name=f"memB{i}")
        rmem = {0: state.tile([128, WPAD], F32R, tag="rmemA", name="rmemA"),
                1: state.tile([128, WPAD], F32R, tag="rmemB", name="rmemB")}
        u = {0: state.tile([128, TOK], F32R, tag="uA", name="uA"),
             1: state.tile([64, TOK], F32R, tag="uB", name="uB")}
        for t in list(mem.values()) + list(rmem.values()):
            p = t.shape[0]
            nc.vector.tensor_copy(t[:], zf32[0:p, :])

        # --- input transform: x[b,l,c] -> mem[0] = [c, 4+l*4+b] ----------
        for b in range(BLOC):
            xb = tmp.tile([L, C], F32, tag="xload")
            nc.sync.dma_start(xb[:], x_d[b])
            for ci, (c0, cl) in enumerate(((0, 128), (128, 64))):
                ps = psum.tile([cl, L], F32, tag=f"tp{ci}")
                nc.tensor.transpose(ps[:], xb[:, c0:c0 + cl], ident[:])
                dst = mem[0, ci][0:cl, 4 + b: 4 + b + 4 * L: 4]
                nc.vector.tensor_copy(dst, ps[:])
        # initial shifted duplicate (partitions 64:128 = B shifted one l)
        nc.vector.tensor_copy(mem[0, 1][64:128, 0:TOK],
                              mem[0, 1][0:64, 4:4 + TOK])

        # --- recurrence ---------------------------------------------------
        cur = 0
        for t in range(steps):
            mA, mB = mem[cur, 0], mem[cur, 1]
            nA, nB = mem[1 - cur, 0], mem[1 - cur, 1]

            ps = {g: psum.tile([GW[g], TOK], F32, tag=f"ps{g}", name=f"ps{g}")
                  for g in GROUPS}

            def conv_mm(g, src0, src1, order):
                """order: subset of chunks to emit now.
                A0/A1/A2: cinA taps; Bp: packed cinB taps 0,1; B2: padded tap2."""
                for ch in order:
                    if ch[0] == "A":
                        k = int(ch[1])
                        nc.tensor.matmul(ps[g][:], wA[g, k][:],
                                         src0[:, k * 4: k * 4 + TOK],
                                         start=(k == 0), stop=False)
                    elif ch == "Bp":
                        nc.tensor.matmul(ps[g][:], wp[g][:], src1[:, 0:TOK],
                                         start=False, stop=False)
                    else:  # B2
                        nc.tensor.matmul(ps[g][:], w2[g][:], src1[:, 8:8 + TOK],
                                         start=False, stop=True)

            # RG convs; A-chunks first (B state + its dup land late in the
            # previous step's tail), rA completes first for the sigmoid
            conv_mm("rgB", mA, mB, ("A0", "A1", "A2"))
            conv_mm("rA", mA, mB, ("A0", "A1", "A2"))
            conv_mm("gA", mA, mB, ("A0", "A1", "A2"))
            conv_mm("rgB", mA, mB, ("Bp", "B2"))
            conv_mm("rA", mA, mB, ("Bp", "B2"))
            conv_mm("gA", mA, mB, ("Bp", "B2"))

            # the rgB path is the long pole: sigmoid(rgB) -> rmemB -> dup ->
            # cand B-chunks -> tanh gates the whole combine, so it goes first
            sig = {}
            for g in ("rgB", "rA", "gA"):
                s = act.tile([128, TOK], F32R, tag=f"s{g}")
                nc.scalar.activation(s[:], ps[g][:], AF.Sigmoid,
                                     bias=bias[g][:, 0:1])
                sig[g] = s

            # rmem = sigmoid(reset) * mem   (reset-B lives in sig[rgB][64:128];
            # mB rows 64:128 are the token-aligned dup, so compute at base 64
            # and copy down: TT operands must share a start partition)
            nc.vector.tensor_mul(rmem[1][64:128, 0:TOK], sig["rgB"][64:128],
                                 mB[64:128, 0:TOK])
            nc.vector.tensor_copy(rmem[1][0:64, 4:4 + TOK], rmem[1][64:128, 0:TOK])
            nc.vector.tensor_mul(rmem[0][:, 4:4 + TOK], sig["rA"][:],
                                 mA[:, 4:4 + TOK])
            nc.vector.tensor_mul(u[0][:], sig["gA"][:], mA[:, 0:TOK])
            nc.vector.tensor_mul(u[1][:], sig["rgB"][0:64], mB[0:64, 0:TOK])

            # cand conv
            conv_mm("cA", rmem[0], rmem[1], ("A0", "A1", "A2"))
            conv_mm("cB", rmem[0], rmem[1], ("A0", "A1", "A2"))
            conv_mm("cA", rmem[0], rmem[1], ("Bp", "B2"))
            conv_mm("cB", rmem[0], rmem[1], ("Bp", "B2"))

            cdA = act.tile([128, TOK], F32R, tag="cdA")
            nc.scalar.activation(cdA[:], ps["cA"][:], AF.Tanh,
                                 bias=bias_cA[:, 0:1])
            cdB = act.tile([64, TOK], F32R, tag="cdB")
            nc.scalar.activation(cdB[:], ps["cB"][:], AF.Tanh,
                                 bias=bias_cB[:, 0:1])

            # mem_next = u - (gate-1)*cand; A half first (next step's RG
            # A-chunks wait on it), B tail trails under the next RG phase
            qA = tmp.tile([128, TOK], F32R, tag="qA", name="qA")
            nc.vector.scalar_tensor_tensor(
                qA[:], sig["gA"][:], 1.0, cdA[:],
                op0=ALU.subtract, op1=ALU.mult)
            subAi = nc.vector.tensor_sub(nA[:, 4:4 + TOK], u[0][:], qA[:])
            qB = tmp.tile([64, TOK], F32R, tag="qB", name="qB")
            qBi = nc.vector.scalar_tensor_tensor(
                qB[:], sig["rgB"][0:64], 1.0, cdB[:],
                op0=ALU.subtract, op1=ALU.mult)
            tile.add_dep_helper(qBi.ins, subAi.ins, sync=False,
                                reason="A tail before B tail")
            nc.vector.tensor_sub(nB[0:64, 4:4 + TOK], u[1][:], qB[:])
            nc.vector.tensor_copy(nB[64:128, 0:TOK], nB[0:64, 4:4 + TOK])

            # keep-warm: fill the PE tail (waiting on the A combine) so HAM
            # doesn't re-throttle; ~1us gap is safe, ~2.4us is not
            for dk in range(4):
                dummy = psum.tile([128, TOK], F32, tag="dm", name=f"dm{t}_{dk}")
                nc.tensor.matmul(dummy[:], wA["rA", dk % 3][:], mA[:, 0:TOK],
                                 start=True, stop=True)

            cur = 1 - cur

        # --- output transform: mem[cur] -> out[b,l,c] ---------------------
        for b in range(BLOC):
            osb = tmp.tile([L, C], F32, tag="oload")
            for ci, (c0, cl) in enumerate(((0, 128), (128, 64))):
                ps = psum.tile([L, cl], F32R, tag=f"tp{ci}")
                nc.tensor.transpose(ps[:], mem[cur, ci][0:cl, 4 + b: 4 + b + 4 * L: 4],
                                    identr[0:cl, 0:cl])
                nc.vector.tensor_copy(osb[:, c0:c0 + cl], ps[:])
            nc.sync.dma_start(out_d[b], osb[:])

    nc.compile()
    return nc


_built = {}


def _get(steps=STEPS):
    if steps not in _built:
        _built[steps] = build(steps)
    return _built[steps]


def kernel(x, w_reset, b_reset, w_gate, b_gate, w_cand, b_cand, steps=STEPS,
           trace=False):
    nc = _get(steps)
    ident = np.eye(128, dtype=np.float32)
    base = {"w_reset": np.asarray(w_reset, np.float32),
            "b_reset": np.asarray(b_reset, np.float32),
            "w_gate": np.asarray(w_gate, np.float32),
            "b_gate": np.asarray(b_gate, np.float32),
            "w_cand": np.asarray(w_cand, np.float32),
            "b_cand": np.asarray(b_cand, np.float32),
            "ident": ident}
    x = np.asarray(x, np.float32)
    in_maps = [dict(base, x=np.ascontiguousarray(x[i * BLOC:(i + 1) * BLOC]))
               for i in range(NCORES)]
    res = run_bass_kernel_spmd(nc, in_maps, core_ids=list(range(NCORES)),
                               trace=trace)
    out = np.concatenate([res.results[i]["out"] for i in range(NCORES)], axis=0)
    if trace:
        return out, res
    return out


if __name__ == "__main__":
    rng = np.random.default_rng(0)
    scale = 1.0 / np.sqrt(3 * C)
    ins = {
        "x": rng.standard_normal((B, L, C), dtype=np.float32),
        "w_reset": (rng.standard_normal((3, C, C)) * scale).astype(np.float32),
        "b_reset": np.full(C, 0.5, np.float32),
        "w_gate": (rng.standard_normal((3, C, C)) * scale).astype(np.float32),
        "b_gate": np.full(C, 0.7, np.float32),
        "w_cand": (rng.standard_normal((3, C, C)) * scale).astype(np.float32),
        "b_cand": np.zeros(C, np.float32),
    }
    out = kernel(**ins, steps=2)
    print("smoke ok", out.shape, out.dtype)
